# revision 1
# baseline (speedup 1.0000x reference)
"""Trainium2 Bass kernel for nn_Backbone_20332375179599.

For binary position tensors top/left [B, L, N] and an indicator [B, L]:

    D[b,i,j] = sum_n |top[b,i,n]-top[b,j,n]| + sum_n |left[b,i,n]-left[b,j,n]|
    out = (D * (1-cls_m) + 100*pad_m) * (1 + 100*(1 - sep_i*sep_j))

Because the positions are binary, |a-b| = a(1-2b) + b, so with
u = [top; left] (K = 576 rows, one column per token) and w = 1-2u:

    D[i,j] = sum_k u[k,i]*w[k,j] + sum_k 1*u[k,j]

i.e. D lands directly in PSUM from two matmul sets over the same K
chunks (lhsT = query columns of u / all-ones), run in fp8 DoubleRow
(values 0/1/-1 are fp8-exact; fp32 PSUM accumulation keeps everything
integer-exact). The masks are rank-2 and rank-4 outer products of
per-token indicator vectors, each one tiny bf16 matmul into its own
PSUM bank:

    out = D * B2 + B3
    B2[i,j] = r_i r_j (101 - 100 q_i q_j)          (r = not-CLS, q = SEP)
    B3[i,j] = (100 - 100 p_i p_j)(101 - 100 q_i q_j)   (p = not-PAD)

B3 is shipped scaled by 1/100 (entries then bf16-exact) and restored by
the fused scalar_tensor_tensor epilogue op.

Sharding: 8 cores = 2 batches x 4 query-row blocks of 128; each core
computes a [128, 512] slab of the [B, 512, 512] output. Key columns are
rotated per core so its own query block sits at columns 0:128 (SPMD
cores share one program, so operand slices must be static); the host
un-rotates the output columns.
"""

import sys

sys.path.insert(0, "/opt/trn_rl_repo")

import numpy as np
import ml_dtypes

B, L, NNODE = 2, 512, 288
KTOT = 2 * NNODE  # 576
KCH = 96  # chunk height: 576 = 6 x 96, so no zero-padding rows are shipped
NCHUNK = 6
N_CORES = 8
CORES_PER_BATCH = 4
QROWS = L // CORES_PER_BATCH  # 128
H0 = 256  # column-split point for the distance banks / w tensors / stores
H1 = L - H0

_CACHE = {}


def _build_module():
    import concourse.mybir as mybir
    import concourse.tile as tile
    from concourse import bacc

    f32 = mybir.dt.float32
    bf16 = mybir.dt.bfloat16
    fp8 = mybir.dt.float8e4

    nc = bacc.Bacc(
        "TRN2", target_bir_lowering=False, debug=False, num_devices=N_CORES
    )

    # Columns are key-rotated per core so this core's 128 query tokens sit
    # at columns 0:128 (the host un-rotates the output). u and w are
    # separate DMAs so the u-side matmuls start before w lands.
    # partition-major 3D layout: [p, c, n] with full-K row = c*KCH + p, so
    # each DMA descriptor moves one partition's contiguous bytes
    u_d = nc.dram_tensor("u", [KCH, NCHUNK, L], fp8, kind="ExternalInput").ap()
    w0_d = nc.dram_tensor("w0", [KCH, NCHUNK, H0], fp8, kind="ExternalInput").ap()
    w1_d = nc.dram_tensor("w1", [KCH, NCHUNK, H1], fp8, kind="ExternalInput").ap()
    # mask operands, 6 rows x [rhs L | b3-lhsT QROWS | b2-lhsT QROWS].
    # b2 reads rows 0:2; b3 reads all 6 rows but its lhsT columns are zero
    # in rows 0:2, so those rows contribute nothing — this keeps both
    # matmul operands at base partition 0 (constraint: 0/32/64) without
    # shipping 30 rows of partition padding.
    m_d = nc.dram_tensor("m", [6, L + 2 * QROWS], bf16, kind="ExternalInput").ap()
    out_d = nc.dram_tensor("out", [QROWS, L], f32, kind="ExternalOutput").ap()

    with tile.TileContext(nc) as tc:
        _kernel_body(tc, mybir, out_d, u_d, w0_d, w1_d, m_d)

    nc.compile()
    return nc


def _kernel_body(tc, mybir, out_d, u_d, w0_d, w1_d, m_d):
    nc = tc.nc
    f32 = mybir.dt.float32
    bf16 = mybir.dt.bfloat16
    fp8 = mybir.dt.float8e4
    DR = mybir.MatmulPerfMode.DoubleRow
    Alu = mybir.AluOpType
    UW = L + QROWS  # 640

    with (
        tc.tile_pool(name="sb", bufs=1) as sb,
        tc.tile_pool(name="ps", bufs=1, space="PSUM") as ps,
    ):
        # One DMA per DRAM tensor, issued u -> m -> w: HWDGE and the DMA
        # engines serialize, u gates the first matmuls (so it goes first),
        # m's transfer is tiny and slots between u and w, and w is only
        # needed by the second matmul set. u/w land chunk-major: chunk c is
        # u_sb[:, c, :], full-K row index = c*KCH + p.
        u_sb = sb.tile([KCH, NCHUNK, L], fp8, tag="u")
        w0_sb = sb.tile([KCH, NCHUNK, H0], fp8, tag="w0")
        w1_sb = sb.tile([KCH, NCHUNK, H1], fp8, tag="w1")
        m_sb = sb.tile([6, L + 2 * QROWS], bf16, tag="m")
        nc.sync.dma_start(u_sb[:, :, :], u_d[:, :, :])
        nc.sync.dma_start(m_sb[:, :], m_d[:, :])
        nc.sync.dma_start(w0_sb[:, :, :], w0_d[:, :, :])
        nc.sync.dma_start(w1_sb[:, :, :], w1_d[:, :, :])

        ones_sb = sb.tile([KCH, 2, QROWS], fp8, tag="ones")
        nc.vector.memset(ones_sb[:, :, :], 1.0)

        # distance accumulates into two column-split banks so the first
        # part's epilogue + store overlap the second part's matmuls
        psum_d0 = ps.tile([QROWS, H0], f32, tag="psum_d0")
        psum_d1 = ps.tile([QROWS, H1], f32, tag="psum_d1")
        psum_b2 = ps.tile([QROWS, L], f32, tag="psum_b2")
        psum_b3 = ps.tile([QROWS, L], f32, tag="psum_b3")

        # D = sum_c [ ones.T @ u_c  +  uq_c.T @ w_c ], fp8 DoubleRow on
        # 96-row chunk pairs (0,1), (2,3), (4,5) — no leftover chunk. The
        # query block is columns 0:QROWS of the rotated u. PE order: the
        # u-only set first (u lands first), then the mask matmuls (m lands
        # while setB runs), then the w set (w lands last).
        halves = []
        for h, pd in ((0, psum_d0), (1, psum_d1)):
            hs = slice(0, H0) if h == 0 else slice(H0, L)
            halves.append((hs, pd))
            nc.tensor.matmul(
                pd[:, :], ones_sb[:, :, :], u_sb[:, 0:2, hs],
                start=True, stop=False, perf_mode=DR,
            )

        # mask matmuls slotted here (m has landed by now) so the B2 copy —
        # done in halves, h0 first — is off the epilogue's critical path
        nc.tensor.matmul(
            psum_b2[:, :], m_sb[0:2, UW : UW + QROWS], m_sb[0:2, :L],
            start=True, stop=True,
        )
        nc.tensor.matmul(
            psum_b3[:, :], m_sb[0:6, L:UW], m_sb[0:6, :L], start=True, stop=True
        )
        # B2 to SBUF so the epilogue ops each have at most one PSUM operand.
        b2_sb = sb.tile([QROWS, L], f32, tag="b2_sb")
        nc.scalar.copy(b2_sb[:, :H0], psum_b2[:, :H0])
        nc.scalar.copy(b2_sb[:, H0:], psum_b2[:, H0:])

        for hs, pd in halves:
            nc.tensor.matmul(
                pd[:, :], ones_sb[:, :, :], u_sb[:, 2:4, hs],
                start=False, stop=False, perf_mode=DR,
            )
            nc.tensor.matmul(
                pd[:, :], ones_sb[:, :, :], u_sb[:, 4:6, hs],
                start=False, stop=False, perf_mode=DR,
            )

        # w-side matmuls: all of half 0 first so its bank closes ~3 matmuls
        # before half 1's, overlapping the h0 epilogue and store with the
        # h1 matmuls.
        for h, pd in ((0, psum_d0), (1, psum_d1)):
            hs = slice(0, H0) if h == 0 else slice(H0, L)
            wh = w0_sb if h == 0 else w1_sb
            hw_ = H0 if h == 0 else L - H0
            t_sb = sb.tile([QROWS, hw_], f32, tag=f"t_sb{h}")
            o_sb = sb.tile([QROWS, hw_], f32, tag=f"o_sb{h}")
            nc.tensor.matmul(
                pd[:, :], u_sb[:, 0:2, :QROWS], wh[:, 0:2, :],
                start=False, stop=False, perf_mode=DR,
            )
            nc.tensor.matmul(
                pd[:, :], u_sb[:, 2:4, :QROWS], wh[:, 2:4, :],
                start=False, stop=False, perf_mode=DR,
            )
            nc.tensor.matmul(
                pd[:, :], u_sb[:, 4:6, :QROWS], wh[:, 4:6, :],
                start=False, stop=True, perf_mode=DR,
            )
            nc.vector.tensor_tensor(
                out=t_sb[:, :], in0=pd[:, :], in1=b2_sb[:, hs], op=Alu.mult
            )
            # o = 100 * B3' + t   (B3 was shipped scaled by 1/100 to stay
            # bf16-exact: values {±1, ±100, ±101})
            nc.vector.scalar_tensor_tensor(
                out=o_sb[:, :], in0=psum_b3[:, hs], scalar=100.0,
                in1=t_sb[:, :], op0=Alu.mult, op1=Alu.add,
            )
            nc.sync.dma_start(out_d[:, hs], o_sb[:, :])


def _get_nc():
    if "nc" not in _CACHE:
        _CACHE["nc"] = _build_module()
    return _CACHE["nc"]


def _pack_m(m2, m3):
    # m2/m3: [rows, L + QROWS] = [rhs | lhsT]. Pack into [6, L + 2*QROWS]:
    # rows 0:2 = b2 (lhsT in the third column block, b3-lhsT block zeroed),
    # rows 2:6 = b3 (lhsT in the second column block).
    Lc = m2.shape[1] - QROWS
    m = np.zeros((6, Lc + 2 * QROWS), m2.dtype)
    m[0:2, :Lc] = m2[:, :Lc]
    m[0:2, Lc + QROWS :] = m2[:, Lc:]
    m[2:6, :Lc] = m3[:, :Lc]
    m[2:6, Lc : Lc + QROWS] = m3[:, Lc:]
    return m


def _make_in_maps(entire_top, entire_left, indicator):
    bf16 = ml_dtypes.bfloat16
    fp8 = ml_dtypes.float8_e4m3
    in_maps = []
    per_batch = {}
    for b in range(B):
        u = np.concatenate([entire_top[b], entire_left[b]], axis=1).T.astype(
            np.float32
        )  # [KTOT, L], no padding: KTOT = 6 chunks of KCH
        w = 1.0 - 2.0 * u
        ind = np.asarray(indicator[b])
        cls = ind == -1
        pad = ind == 0
        sep = (ind > 0) & (ind % 2 == 1)
        r = (~cls).astype(np.float32)
        p = (~pad).astype(np.float32)
        q = sep.astype(np.float32)
        ones = np.ones(L, np.float32)
        # rows: [rhs over keys | lhsT over this core's queries]
        m2_rhs = np.stack([r, r * q])  # [2, L]
        m2_lhs = np.stack([101.0 * r, -100.0 * r * q])  # [2, L] -> slice
        # B3 shipped scaled by 1/100 so every entry is bf16-exact; the
        # epilogue multiplies the bank by 100.
        m3_rhs = np.stack([ones, q, p, p * q])  # [4, L]
        m3_lhs = np.stack([101.0 * ones, -100.0 * q, -101.0 * p, 100.0 * p * q])
        per_batch[b] = (
            u.astype(fp8),
            w.astype(fp8),
            m2_rhs.astype(bf16),
            m2_lhs.astype(bf16),
            m3_rhs.astype(bf16),
            m3_lhs.astype(bf16),
        )

    for c in range(N_CORES):
        b, qi = c // CORES_PER_BATCH, c % CORES_PER_BATCH
        u, w, m2_rhs, m2_lhs, m3_rhs, m3_lhs = per_batch[b]
        k = qi * QROWS
        rot = lambda a: np.ascontiguousarray(np.roll(a, -k, axis=-1))
        m2r, m2l = rot(m2_rhs), rot(m2_lhs)
        m3r, m3l = rot(m3_rhs), rot(m3_lhs)
        pm = lambda a: np.ascontiguousarray(
            a.reshape(NCHUNK, KCH, -1).transpose(1, 0, 2)
        )
        wr = rot(w)
        in_maps.append(
            {
                "u": pm(rot(u)),
                "w0": pm(wr[:, :H0]),
                "w1": pm(wr[:, H0:]),
                "m": _pack_m(
                    np.concatenate([m2r, m2l[:, :QROWS]], axis=1),
                    np.concatenate([m3r, m3l[:, :QROWS]], axis=1),
                ),
            }
        )
    return in_maps


def run(entire_top, entire_left, indicator, trace=False):
    from concourse import bass_utils

    nc = _get_nc()
    in_maps = _make_in_maps(entire_top, entire_left, indicator)
    res = bass_utils.run_bass_kernel_spmd(
        nc, in_maps, core_ids=list(range(N_CORES)), trace=trace
    )
    out = np.empty((B, L, L), np.float32)
    for c in range(N_CORES):
        b, qi = c // CORES_PER_BATCH, c % CORES_PER_BATCH
        # columns were key-rotated by -k on the way in; rotate back
        out[b, qi * QROWS : (qi + 1) * QROWS, :] = np.roll(
            res.results[c]["out"], qi * QROWS, axis=-1
        )
    return out, res


def kernel(entire_top, entire_left, indicator):
    out, _ = run(
        np.asarray(entire_top, dtype=np.float32),
        np.asarray(entire_left, dtype=np.float32),
        np.asarray(indicator),
    )
    return out

